# revision 1
# baseline (speedup 1.0000x reference)
"""BlockGRUCell fused Trainium2 kernel.

Sharding: data-parallel over batch across 8 NeuronCores (weights replicated).
Dataflow is fully transposed ([features, batch] on-chip): every matmul uses
the weight matrix in natural [in, out] layout as the stationary lhsT and the
transposed activations [in, batch] as the moving rhs, producing the next
layer's transposed activations directly. LayerNorm reductions run over the
partition axis via ones-vector matmuls; per-batch stats are broadcast back
across partitions with a rank-1 matmul.
"""
import numpy as np
from contextlib import ExitStack

import concourse.bass as bass
import concourse.tile as tile
from concourse import bacc, mybir
from concourse._compat import with_exitstack
from concourse.bass_utils import run_bass_kernel_spmd

B, D, S, A, H, G = 2048, 4096, 1024, 1024 // 8, 1024, 8
A = 128
DG = D // G            # 512
ING = DG + 3 * H       # 3584
NCORES = 8
BC = B // NCORES       # 256 batch rows per core
EPS = 1e-5
USE_SILU = True  # stage-1 only; CoreSim tests set False

F32 = mybir.dt.float32
F32R = mybir.dt.float32r
AF = mybir.ActivationFunctionType
OP = mybir.AluOpType

# vecs column layout (all per-partition tiled: col j holds v[j*128 + p])
NB_H = H // 128        # 8
NB_D = D // 128        # 32
NB_Z = 3 * D // 128    # 96
C_BD, C_GD, C_BED = 0, NB_H, 2 * NB_H
C_BS, C_GS, C_BES = 3 * NB_H, 4 * NB_H, 5 * NB_H
C_BA, C_GA, C_BEA = 6 * NB_H, 7 * NB_H, 8 * NB_H
C_DYNB, C_GDYN, C_BEDYN = 9 * NB_H, 9 * NB_H + NB_D, 9 * NB_H + 2 * NB_D
C_GRUB = 9 * NB_H + 3 * NB_D
NV = C_GRUB + NB_Z     # 264


@with_exitstack
def _emit(ctx: ExitStack, tc: tile.TileContext, ins: dict, outT: bass.AP):
    nc = tc.nc
    ctx.enter_context(nc.allow_low_precision(
        reason="float32r outputs are bit-identical fp32; needed as matmul operands"))

    persist = ctx.enter_context(tc.tile_pool(name="persist", bufs=1))
    sqp = ctx.enter_context(tc.tile_pool(name="sqp", bufs=2))
    small = ctx.enter_context(tc.tile_pool(name="small", bufs=1))
    consts = ctx.enter_context(tc.tile_pool(name="consts", bufs=1))
    mm_ps = ctx.enter_context(tc.tile_pool(name="mm_ps", bufs=1, space="PSUM"))
    st_ps = tc.alloc_tile_pool(name="st_ps", bufs=1, space="PSUM")
    bc_ps = tc.alloc_tile_pool(name="bc_ps", bufs=1, space="PSUM")
    wpool = tc.alloc_tile_pool(name="wpool", bufs=6)
    s1pool = tc.alloc_tile_pool(name="s1pool", bufs=1)
    s1upool = tc.alloc_tile_pool(name="s1upool", bufs=2)

    # ---- small constants ----
    vecs_sb = persist.tile([128, NV], F32, name="vecs")
    nc.sync.dma_start(out=vecs_sb, in_=ins["vecs"])
    ones_col_f = consts.tile([128, 1], F32)
    nc.vector.memset(ones_col_f, 1.0)
    ones_col = consts.tile([128, 1], F32R)
    nc.vector.tensor_copy(ones_col, ones_col_f)
    ones_row_f = consts.tile([1, 128], F32)
    nc.vector.memset(ones_row_f, 1.0)
    ones_row = consts.tile([1, 128], F32R)
    nc.vector.tensor_copy(ones_row, ones_row_f)
    eps_t = consts.tile([1, 1], F32)
    nc.vector.memset(eps_t, EPS)

    x_sb = persist.tile([128, 24, BC], F32R, name="x")

    mm_tags = [f"up{i}" for i in range(4)]

    def mm_tile(i):
        return mm_ps.tile([128, 2 * BC], F32, name=mm_tags[i % 4])[:, :BC]

    z_state = {}

    def z_pair(i):
        tags = [(mm_ps, t) for t in mm_tags]
        if "z2" in z_state:
            tags += [(z_state["z2"], f"z2_{j}") for j in range(4)]
        pool, t = tags[i % len(tags)]
        return pool.tile([128, 2 * BC], F32, name=t)

    def stats_finish(ssum, ssq, nfeat):
        """Broadcast LN stats: returns (meanB2, rstdB2) [128, 2*BC] SBUF tiles
        with the per-batch stat duplicated in both halves (for paired apply)."""
        mr2 = small.tile([1, 4 * BC], F32R, name="mr2")
        mean, mean_b = mr2[:, :BC], mr2[:, BC:2 * BC]
        rstd, rstd_b = mr2[:, 2 * BC:3 * BC], mr2[:, 3 * BC:]
        nc.vector.tensor_scalar_mul(mean, ssum, 1.0 / nfeat)
        nc.vector.tensor_copy(mean_b, mean)
        m2 = small.tile([1, BC], F32, name="m2")
        nc.vector.tensor_mul(m2, mean, mean)
        var = small.tile([1, BC], F32, name="var")
        nc.vector.tensor_scalar(var, ssq, 1.0 / nfeat, None, OP.mult, OP.bypass)
        nc.vector.tensor_sub(var, var, m2)
        std = small.tile([1, BC], F32, name="std")
        nc.scalar.activation(std, var, AF.Sqrt, bias=eps_t, scale=1.0)
        nc.vector.reciprocal(rstd, std)
        nc.vector.tensor_copy(rstd_b, rstd)
        bc0 = bc_ps.tile([128, 2 * BC], F32, name="bc0")
        nc.tensor.matmul(bc0, ones_row, mr2[:, :2 * BC], start=True, stop=True)
        bc1 = bc_ps.tile([128, 2 * BC], F32, name="bc1")
        nc.tensor.matmul(bc1, ones_row, mr2[:, 2 * BC:], start=True, stop=True)
        mr0 = sqp.tile([128, 2 * BC], F32, name="mr0")
        nc.vector.tensor_copy(mr0, bc0)
        mr1 = sqp.tile([128, 2 * BC], F32, name="mr1")
        nc.vector.tensor_copy(mr1, bc1)
        return mr0, mr1

    def ln_apply(u_sb_pair, meanB2, rstdB2, g_cols, be_cols, dst_fn, pair_list):
        """Paired LN apply: dst = silu(((u - m) * r) * gamma + beta).

        u_sb_pair(i) -> [128, 2*BC] view of feature tiles (i, i+1).
        Sub/mul run on [128, 512] pairs; the per-tile gamma/beta + silu run on
        the ACT LUT per half (one function -> one table load)."""
        for i in pair_list:
            tp = sqp.tile([128, 2 * BC], F32, name="tp")
            nc.vector.tensor_sub(tp, u_sb_pair(i), meanB2)
            nc.vector.tensor_mul(tp, tp, rstdB2)
            for h in (0, 1):
                idx = i + h
                gcol = vecs_sb[:, g_cols + idx:g_cols + idx + 1]
                becol = vecs_sb[:, be_cols + idx:be_cols + idx + 1]
                th = tp[:, h * BC:(h + 1) * BC]
                if USE_SILU:
                    nc.scalar.activation(dst_fn(idx), th, AF.Silu,
                                         bias=becol, scale=gcol)
                else:
                    sg = sqp.tile([128, BC], F32, name="sg")
                    nc.scalar.activation(sg, th, AF.Sigmoid, bias=becol, scale=gcol)
                    nn = sqp.tile([128, BC], F32, name="nn")
                    nc.gpsimd.tensor_scalar(nn, th, gcol, becol, OP.mult, OP.add)
                    nc.gpsimd.tensor_mul(dst_fn(idx), nn, sg)

    # ================= stage 1: three input projections =================
    deterT_sb = persist.tile([128, NB_D, BC], F32R, name="deterT")
    s1_rhs = {}

    def load_s1_rhs(wname):
        # emitted per-tensor so act loads sit just ahead of their weight
        # panels in the HWDGE FIFO (no head-of-line block of first matmuls)
        if wname == "W_a":
            t = s1pool.tile([128, 1, BC], F32R, name="actionT")
            nc.sync.dma_start(out=t, in_=ins["actionT"].bitcast(F32R)
                              .rearrange("(t p) b -> p t b", p=128))
        elif wname == "W_s":
            t = s1pool.tile([128, S // 128, BC], F32R, name="stochT")
            nc.sync.dma_start(out=t, in_=ins["stochT"].bitcast(F32R)
                              .rearrange("(t p) b -> p t b", p=128))
        else:
            t = deterT_sb
            _dT = ins["deterT"].bitcast(F32R).rearrange("(t p) b -> p t b", p=128)
            for q in range(4):
                nc.sync.dma_start(out=t[:, q * 8:(q + 1) * 8, :],
                                  in_=_dT[:, q * 8:(q + 1) * 8, :])
        return t

    stage1 = [
        ("W_a", 1, C_BA, C_GA, C_BEA, 16),
        ("W_s", S // 128, C_BS, C_GS, C_BES, 8),
        ("W_d", NB_D, C_BD, C_GD, C_BED, 0),
    ]
    for wname, KT, bcol, gcol, becol, xoff in stage1:
        Wap = ins[wname]
        rhs_sb = load_s1_rhs(wname)
        u_sb = s1upool.tile([128, NB_H, BC], F32R, name="u_sb")
        ssum = st_ps.tile([1, BC], F32, name="ssum")
        ssq = st_ps.tile([1, BC], F32, name="ssq")
        for c in range(2):  # H in two chunks of 4 m-tiles
            psums = [mm_tile(m) for m in range(4)]
            KG = (KT + 3) // 4
            for kg in range(KG):
                nk = min(4, KT - kg * 4)
                wp = wpool.tile([128, 4, 512], F32R, name="wp")
                nc.sync.dma_start(
                    out=wp[:, :nk, :],
                    in_=Wap.bitcast(F32R)[kg * 512:kg * 512 + nk * 128,
                                          c * 512:(c + 1) * 512]
                    .rearrange("(kk p) n -> p kk n", p=128),
                )
                for k4 in range(nk):
                    k = kg * 4 + k4
                    for m in range(4):
                        nc.tensor.matmul(
                            psums[m],
                            wp[:, k4, m * 128:(m + 1) * 128],
                            rhs_sb[:, k, :],
                            start=(k == 0), stop=(k == KT - 1),
                        )
            for m in range(4):
                mt = c * 4 + m
                ut = u_sb[:, mt, :]
                nc.vector.tensor_scalar_add(ut, psums[m],
                                            vecs_sb[:, bcol + mt:bcol + mt + 1])
                usq = sqp.tile([128, BC], F32R, name="usq")
                nc.vector.tensor_mul(usq, ut, ut)
                nc.tensor.matmul(ssum, ones_col, ut,
                                 start=(mt == 0), stop=(mt == NB_H - 1))
                nc.tensor.matmul(ssq, ones_col, usq,
                                 start=(mt == 0), stop=(mt == NB_H - 1))
        meanB2, rstdB2 = stats_finish(ssum, ssq, H)
        ln_apply(lambda i, u=u_sb: u[:, i:i + 2, :].rearrange("p a b -> p (a b)"),
                 meanB2, rstdB2, gcol, becol,
                 lambda idx, xoff=xoff: x_sb[:, xoff + idx, :],
                 [0, 2, 4, 6])
    s1upool.release()
    s1pool.release()

    # ================= stage 2: block-diagonal dyn layer =================
    dynW = ins["dyn_W"]  # [G, ING, DG]
    y_sb = persist.tile([128, NB_D, BC], F32R, name="y_sb")
    yssum = st_ps.tile([1, BC], F32, name="ssum")
    yssq = st_ps.tile([1, BC], F32, name="ssq")
    KT2 = ING // 128  # 28

    def rhs2(g, k):
        return deterT_sb[:, g * 4 + k, :] if k < 4 else x_sb[:, k - 4, :]

    for g in range(G):
        psums = [mm_tile(m) for m in range(4)]
        for kg in range(7):
            wp = wpool.tile([128, 4, 512], F32R, name="wp")
            nc.sync.dma_start(
                out=wp,
                in_=dynW.bitcast(F32R)[g, kg * 512:(kg + 1) * 512, :]
                .rearrange("(kk p) n -> p kk n", p=128),
            )
            for k4 in range(4):
                k = kg * 4 + k4
                for m in range(4):
                    nc.tensor.matmul(
                        psums[m],
                        wp[:, k4, m * 128:(m + 1) * 128],
                        rhs2(g, k),
                        start=(k == 0), stop=(k == KT2 - 1),
                    )
        for m in range(4):
            ft = g * 4 + m
            yt = y_sb[:, ft, :]
            nc.vector.tensor_scalar_add(yt, psums[m],
                                        vecs_sb[:, C_DYNB + ft:C_DYNB + ft + 1])
            ysq = sqp.tile([128, BC], F32R, name="usq")
            nc.vector.tensor_mul(ysq, yt, yt)
            nc.tensor.matmul(yssum, ones_col, yt,
                             start=(ft == 0), stop=(ft == NB_D - 1))
            nc.tensor.matmul(yssq, ones_col, ysq,
                             start=(ft == 0), stop=(ft == NB_D - 1))
    meanB2, rstdB2 = stats_finish(yssum, yssq, D)
    # bulk apply in stage-3 first-use order; Silu only -> one ACT table load
    _border = [0, 2, 5, 3, 6, 1, 4, 7]
    ln_apply(lambda i: y_sb[:, i:i + 2, :].rearrange("p a b -> p (a b)"),
             meanB2, rstdB2, C_GDYN, C_BEDYN, lambda idx: y_sb[:, idx, :],
             [g * 4 + j for g in _border for j in (0, 2)])
    bc_ps.release()
    st_ps.release()
    z2_ps = tc.alloc_tile_pool(name="z2_ps", bufs=1, space="PSUM")
    z_state["z2"] = z2_ps

    # ================= stage 3: GRU gates + output =================
    # zflat f-tiles: reset tj, cand tj+32, update tj+64; block gf = f*128//1536.
    gruW = ins["gru_W"]  # [G, DG, 3*DG]
    zcnt = 0
    for seg in range(8):  # 4 tj per segment; each offset's 4 f-tiles in one block
        tj0 = seg * 4
        tjs = list(range(tj0, tj0 + 4))
        panels = []
        for off in range(3):
            f0 = tj0 * 128 + off * 4096
            gf, col0 = f0 // 1536, f0 % 1536
            wp = wpool.tile([128, 4, 512], F32R, name="wp")
            nc.sync.dma_start(
                out=wp,
                in_=gruW.bitcast(F32R)[gf, :, col0:col0 + 512]
                .rearrange("(kk p) n -> p kk n", p=128),
            )
            panels.append((wp, gf))

        def zmm(off, tj):
            nonlocal zcnt
            co = (tj - tj0) * 128
            wp, gf = panels[off]
            zp = z_pair(zcnt)[:, :BC]
            zcnt += 1
            for k in range(4):
                nc.tensor.matmul(zp, wp[:, k, co:co + 128], y_sb[:, gf * 4 + k, :],
                                 start=(k == 0), stop=(k == 3))
            return zp

        r, uu, rc = {}, {}, {}
        for tj in tjs:  # reset gate: sigmoid batch
            zp = zmm(0, tj)
            r[tj] = sqp.tile([128, BC], F32, name=f"r{tj % 2}")
            nc.scalar.activation(r[tj], zp, AF.Sigmoid,
                                 bias=vecs_sb[:, C_GRUB + tj:C_GRUB + tj + 1], scale=1.0)
        for tj in tjs:  # update gate: sigmoid batch
            zp = zmm(2, tj)
            uu[tj] = sqp.tile([128, BC], F32, name=f"uu{tj % 2}")
            nc.scalar.activation(uu[tj], zp, AF.Sigmoid,
                                 bias=vecs_sb[:, C_GRUB + 64 + tj:C_GRUB + 64 + tj + 1], scale=1.0)
        for tj in tjs:  # cand pre-act: DVE + gpsimd only
            zp = zmm(1, tj)
            cp = sqp.tile([128, BC], F32, name="cp")
            nc.vector.tensor_scalar_add(cp, zp,
                                        vecs_sb[:, C_GRUB + 32 + tj:C_GRUB + 32 + tj + 1])
            rc[tj] = sqp.tile([128, BC], F32, name=f"rc{tj % 2}")
            nc.gpsimd.tensor_mul(rc[tj], r[tj], cp)
        for tj in tjs:  # tanh via 2*sigmoid(2x)-1; out = uu*(c-d)+d
            ss = sqp.tile([128, BC], F32, name="cc")
            nc.scalar.activation(ss, rc[tj], AF.Sigmoid, bias=0.0, scale=2.0)
            dt_ = deterT_sb[:, tj, :]
            cd = sqp.tile([128, BC], F32, name="cd")
            nc.vector.scalar_tensor_tensor(cd, ss, 2.0, dt_, OP.mult, OP.subtract)
            o = sqp.tile([128, BC], F32, name="o")
            nc.vector.scalar_tensor_tensor(o, cd, 1.0, uu[tj], OP.subtract, OP.mult)
            nc.gpsimd.tensor_add(o, o, dt_)
            nc.sync.dma_start(out=outT[tj * 128:(tj + 1) * 128, :], in_=o)
    z2_ps.release()
    wpool.release()


_CACHE = {}


def _build():
    if "nc" in _CACHE:
        return _CACHE["nc"]
    nc = bacc.Bacc("TRN2", target_bir_lowering=False, debug=False,
                   num_devices=NCORES)
    ins = {}
    for name, shape in [
        ("deterT", [D, BC]), ("stochT", [S, BC]), ("actionT", [A, BC]),
        ("W_d", [D, H]), ("W_s", [S, H]), ("W_a", [A, H]),
        ("dyn_W", [G, ING, DG]), ("gru_W", [G, DG, 3 * DG]),
        ("vecs", [128, NV]),
    ]:
        ins[name] = nc.dram_tensor(name, shape, F32, kind="ExternalInput").ap()
    outT = nc.dram_tensor("outT", [D, BC], F32, kind="ExternalOutput").ap()
    with tile.TileContext(nc) as tc:
        _emit(tc, ins, outT)
    nc.compile()
    _CACHE["nc"] = nc
    return nc


def _col_tile(v):
    """[L] -> [128, L//128] with col t holding v[t*128 + p]."""
    return np.ascontiguousarray(v.reshape(-1, 128).T.astype(np.float32))


def _make_vecs(b_d, g_d, be_d, b_s, g_s, be_s, b_a, g_a, be_a,
               dyn_b, g_dyn, be_dyn, gru_b):
    gru_adj = np.array(gru_b, dtype=np.float32).copy()
    gru_adj[2 * D:] -= 1.0
    cols = [b_d, g_d, be_d, b_s, g_s, be_s, b_a, g_a, be_a,
            dyn_b, g_dyn, be_dyn, gru_adj]
    return np.concatenate([_col_tile(np.asarray(c)) for c in cols], axis=1), gru_adj


def kernel(deter, stoch, action,
           W_d, b_d, g_d, be_d,
           W_s, b_s, g_s, be_s,
           W_a, b_a, g_a, be_a,
           dyn_W, dyn_b, g_dyn, be_dyn,
           gru_W, gru_b):
    nc = _build()

    deterT = np.ascontiguousarray(np.asarray(deter, dtype=np.float32).T)
    stochT = np.ascontiguousarray(np.asarray(stoch, dtype=np.float32).T)
    actionT = np.ascontiguousarray(np.asarray(action, dtype=np.float32).T)
    vecs, gru_adj = _make_vecs(b_d, g_d, be_d, b_s, g_s, be_s, b_a, g_a, be_a,
                               dyn_b, g_dyn, be_dyn, gru_b)
    shared = {
        "W_d": np.ascontiguousarray(np.asarray(W_d, dtype=np.float32)),
        "W_s": np.ascontiguousarray(np.asarray(W_s, dtype=np.float32)),
        "W_a": np.ascontiguousarray(np.asarray(W_a, dtype=np.float32)),
        "dyn_W": np.ascontiguousarray(np.asarray(dyn_W, dtype=np.float32)),
        "gru_W": np.ascontiguousarray(np.asarray(gru_W, dtype=np.float32)),
        "vecs": vecs,
    }
    in_maps = []
    for c in range(NCORES):
        sl = slice(c * BC, (c + 1) * BC)
        m = dict(shared)
        m["deterT"] = np.ascontiguousarray(deterT[:, sl])
        m["stochT"] = np.ascontiguousarray(stochT[:, sl])
        m["actionT"] = np.ascontiguousarray(actionT[:, sl])
        in_maps.append(m)

    import os
    kw = {}
    if os.environ.get("BASS_TMPDIR"):
        kw["tmpdir"] = os.environ["BASS_TMPDIR"]
    res = run_bass_kernel_spmd(nc, in_maps, list(range(NCORES)), **kw)
    global LAST_RES
    LAST_RES = res
    outT = np.concatenate([res.results[c]["outT"] for c in range(NCORES)], axis=1)
    return np.ascontiguousarray(outT.T)


LAST_RES = None



# revision 6
# speedup vs baseline: 1.2397x; 1.2397x over previous
"""BlockGRUCell fused Trainium2 kernel (bf16 matmul inputs).

Sharding: data-parallel over batch across 8 NeuronCores (weights replicated).
Dataflow is fully transposed ([features, batch] on-chip): every matmul uses
the weight matrix in natural [in, out] layout as the stationary lhsT and the
transposed activations [in, batch] as the moving rhs, producing the next
layer's transposed activations directly. All main matmuls run in bf16
(stationary weights bf16 -> Fast Weight Load hides LDWEIGHTS; halves HBM
traffic). LayerNorm stats and the GRU elementwise stay fp32. Weights are
pre-rearranged on the host into [partition, k, n] order so every DMA moves
multi-KB contiguous lines per partition.

PSUM discipline: opening an accumulation chain (start=True, stop later)
zeroes the whole PSUM bank, so at most ONE open chain per bank — every
concurrent chain gets its own [128, 2*BC] tile (one bank), left half used.
"""
import numpy as np
from contextlib import ExitStack

import ml_dtypes

import concourse.bass as bass
import concourse.tile as tile
from concourse import bacc, mybir
from concourse._compat import with_exitstack
from concourse.bass_utils import run_bass_kernel_spmd

B, D, S, H, G = 2048, 4096, 1024, 1024, 8
A = 128
DG = D // G            # 512
ING = DG + 3 * H       # 3584
NCORES = 8
BC = B // NCORES       # 256 batch rows per core
EPS = 1e-5
USE_SILU = True  # stage-1 only; CoreSim tests set False

F32 = mybir.dt.float32
F32R = mybir.dt.float32r
BF16 = mybir.dt.bfloat16
AF = mybir.ActivationFunctionType
OP = mybir.AluOpType
NPBF = ml_dtypes.bfloat16

# vecs column layout (all per-partition tiled: col j holds v[j*128 + p])
NB_H = H // 128        # 8
NB_D = D // 128        # 32
NB_Z = 3 * D // 128    # 96
C_BD, C_GD, C_BED = 0, NB_H, 2 * NB_H
C_BS, C_GS, C_BES = 3 * NB_H, 4 * NB_H, 5 * NB_H
C_BA, C_GA, C_BEA = 6 * NB_H, 7 * NB_H, 8 * NB_H
C_DYNB, C_GDYN, C_BEDYN = 9 * NB_H, 9 * NB_H + NB_D, 9 * NB_H + 2 * NB_D
C_GRUB = 9 * NB_H + 3 * NB_D
NV = C_GRUB + NB_Z     # 264

# stage-3 panel table: panel p = seg*3 + off covers z flat cols
# f0 = seg*512 + off*4096 .. +512 -> block gf = f0 // 1536, col0 = f0 % 1536
PANELS = []
for _seg in range(8):
    for _off in range(3):
        _f0 = _seg * 512 + _off * 4096
        PANELS.append((_f0 // 1536, _f0 % 1536))


@with_exitstack
def _emit(ctx: ExitStack, tc: tile.TileContext, ins: dict, outT: bass.AP):
    nc = tc.nc
    ctx.enter_context(nc.allow_low_precision(
        reason="bf16 matmul operands; fp32 accumulation and LN stats"))

    persist = ctx.enter_context(tc.tile_pool(name="persist", bufs=1))
    sqp = ctx.enter_context(tc.tile_pool(name="sqp", bufs=2))
    small = ctx.enter_context(tc.tile_pool(name="small", bufs=1))
    consts = ctx.enter_context(tc.tile_pool(name="consts", bufs=1))
    mm_ps = ctx.enter_context(tc.tile_pool(name="mm_ps", bufs=1, space="PSUM"))
    st_ps = ctx.enter_context(tc.tile_pool(name="st_ps", bufs=1, space="PSUM"))
    bc_ps = ctx.enter_context(tc.tile_pool(name="bc_ps", bufs=1, space="PSUM"))
    wkpool = ctx.enter_context(tc.tile_pool(name="wkpool", bufs=6))
    s1pool = tc.alloc_tile_pool(name="s1pool", bufs=1)
    s1upool = tc.alloc_tile_pool(name="s1upool", bufs=2)

    # ---- small constants ----
    vecs_sb = persist.tile([128, NV], F32, name="vecs")
    nc.sync.dma_start(out=vecs_sb, in_=ins["vecs"])
    ones_col_f = consts.tile([128, 1], F32)
    nc.vector.memset(ones_col_f, 1.0)
    ones_col = consts.tile([128, 1], F32R)
    nc.vector.tensor_copy(ones_col, ones_col_f)
    ones_row_f = consts.tile([1, 128], F32)
    nc.vector.memset(ones_row_f, 1.0)
    ones_row = consts.tile([1, 128], F32R)
    nc.vector.tensor_copy(ones_row, ones_row_f)
    eps_t = consts.tile([1, 1], F32)
    nc.vector.memset(eps_t, EPS)

    # persistent activations
    x_sb = persist.tile([128, 24, BC], BF16, name="x")          # post-LN stage1
    deterF = persist.tile([128, NB_D, BC], F32, name="deterF")  # fp32 deter
    deterB = persist.tile([128, NB_D, BC], BF16, name="deterB")
    y_u = persist.tile([128, NB_D, BC], F32R, name="y_u")       # pre-LN stage2
    y_bf = persist.tile([128, NB_D, BC], BF16, name="y_bf")     # post-LN stage2

    def mm_tile(j):
        """One full PSUM bank per accumulation chain; left half carries data."""
        return mm_ps.tile([128, 2 * BC], F32, name=f"q{j}")[:, :BC]

    def stats_finish(ssum, ssq, nfeat):
        """Broadcast LN stats: returns (meanB2, rstdB2) [128, 2*BC] PSUM tiles
        with the per-batch stat duplicated in both halves (for paired apply)."""
        mr2 = small.tile([1, 4 * BC], F32R, name="mr2")
        mean, mean_b = mr2[:, :BC], mr2[:, BC:2 * BC]
        rstd, rstd_b = mr2[:, 2 * BC:3 * BC], mr2[:, 3 * BC:]
        nc.vector.tensor_scalar_mul(mean, ssum, 1.0 / nfeat)
        nc.vector.tensor_copy(mean_b, mean)
        m2 = small.tile([1, BC], F32, name="m2")
        nc.vector.tensor_mul(m2, mean, mean)
        var = small.tile([1, BC], F32, name="var")
        nc.vector.tensor_scalar(var, ssq, 1.0 / nfeat, None, OP.mult, OP.bypass)
        nc.vector.tensor_sub(var, var, m2)
        std = small.tile([1, BC], F32, name="std")
        nc.scalar.activation(std, var, AF.Sqrt, bias=eps_t, scale=1.0)
        nc.vector.reciprocal(rstd, std)
        nc.vector.tensor_copy(rstd_b, rstd)
        bc0 = bc_ps.tile([128, 2 * BC], F32, name="bc0")
        nc.tensor.matmul(bc0, ones_row, mr2[:, :2 * BC], start=True, stop=True)
        bc1 = bc_ps.tile([128, 2 * BC], F32, name="bc1")
        nc.tensor.matmul(bc1, ones_row, mr2[:, 2 * BC:], start=True, stop=True)
        return bc0, bc1

    def ln_apply(u_sb_pair, meanB2, rstdB2, g_cols, be_cols, dst_fn, pair_list):
        """Paired LN apply: dst = silu(((u - m) * r) * gamma + beta).

        u_sb_pair(i) -> [128, 2*BC] view of feature tiles (i, i+1); stats are
        read straight from the PSUM broadcast tiles."""
        for i in pair_list:
            tp = sqp.tile([128, 2 * BC], F32, name="tp")
            nc.vector.tensor_sub(tp, u_sb_pair(i), meanB2)
            nc.vector.tensor_mul(tp, tp, rstdB2)
            for h in (0, 1):
                idx = i + h
                gcol = vecs_sb[:, g_cols + idx:g_cols + idx + 1]
                becol = vecs_sb[:, be_cols + idx:be_cols + idx + 1]
                th = tp[:, h * BC:(h + 1) * BC]
                if USE_SILU:
                    nc.scalar.activation(dst_fn(idx), th, AF.Silu,
                                         bias=becol, scale=gcol)
                else:
                    sg = sqp.tile([128, BC], F32, name="sg")
                    nc.scalar.activation(sg, th, AF.Sigmoid, bias=becol, scale=gcol)
                    nn = sqp.tile([128, BC], F32, name="nn")
                    nc.gpsimd.tensor_scalar(nn, th, gcol, becol, OP.mult, OP.add)
                    nc.gpsimd.tensor_mul(dst_fn(idx), nn, sg)

    # ================= stage 1: three input projections =================
    def load_s1_rhs(wname):
        # emitted per-tensor so act loads sit just ahead of their weight
        # panels in the HWDGE FIFO (no head-of-line block of first matmuls)
        if wname == "W_a":
            t = s1pool.tile([128, 1, BC], BF16, name="actionT")
            nc.sync.dma_start(out=t.rearrange("p a b -> p (a b)"),
                              in_=ins["actionT_b"])
        elif wname == "W_s":
            t = s1pool.tile([128, S // 128, BC], BF16, name="stochT")
            nc.sync.dma_start(out=t.rearrange("p a b -> p (a b)"),
                              in_=ins["stochT_b"])
        else:
            t = deterB
            for q in range(4):
                fview = deterF[:, q * 8:(q + 1) * 8, :].rearrange("p a b -> p (a b)")
                nc.sync.dma_start(out=fview,
                                  in_=ins["deterT_r"][:, q * 2048:(q + 1) * 2048])
                nc.gpsimd.tensor_copy(
                    t[:, q * 8:(q + 1) * 8, :].rearrange("p a b -> p (a b)"),
                    fview)
        return t

    stage1 = [
        ("W_a", 1, C_BA, C_GA, C_BEA, 16),
        ("W_s", S // 128, C_BS, C_GS, C_BES, 8),
        ("W_d", NB_D, C_BD, C_GD, C_BED, 0),
    ]
    for wname, KT, bcol, gcol, becol, xoff in stage1:
        Wsrc = ins[wname + "_r"]  # [128, 2*KT*512] bf16: chunk-major, k-major
        rhs_sb = load_s1_rhs(wname)
        u_sb = s1upool.tile([128, NB_H, BC], F32R, name="u_sb")
        ssum = st_ps.tile([1, BC], F32, name="ssum")
        ssq = st_ps.tile([1, BC], F32, name="ssq")
        for c in range(2):  # H in two chunks of 4 m-tiles
            psums = [mm_tile(j) for j in range(4)]
            KG = (KT + 3) // 4
            for kg in range(KG):
                nk = min(4, KT - kg * 4)
                wp = wkpool.tile([128, 4, 512], BF16, name="wk")
                base = c * KT * 512 + kg * 2048
                nc.sync.dma_start(
                    out=wp[:, :nk, :].rearrange("p a b -> p (a b)"),
                    in_=Wsrc[:, base:base + nk * 512],
                )
                for k4 in range(nk):
                    k = kg * 4 + k4
                    for m in range(4):
                        nc.tensor.matmul(
                            psums[m],
                            wp[:, k4, m * 128:(m + 1) * 128],
                            rhs_sb[:, k, :],
                            start=(k == 0), stop=(k == KT - 1),
                        )
            for m in range(4):
                mt = c * 4 + m
                ut = u_sb[:, mt, :]
                nc.vector.tensor_scalar_add(ut, psums[m],
                                            vecs_sb[:, bcol + mt:bcol + mt + 1])
                usq = sqp.tile([128, BC], F32R, name="usq")
                nc.gpsimd.tensor_mul(usq, ut, ut)
                nc.tensor.matmul(ssum, ones_col, ut,
                                 start=(mt == 0), stop=(mt == NB_H - 1))
                nc.tensor.matmul(ssq, ones_col, usq,
                                 start=(mt == 0), stop=(mt == NB_H - 1))
        meanB2, rstdB2 = stats_finish(ssum, ssq, H)
        ln_apply(lambda i, u=u_sb: u[:, i:i + 2, :].rearrange("p a b -> p (a b)"),
                 meanB2, rstdB2, gcol, becol,
                 lambda idx, xoff=xoff: x_sb[:, xoff + idx, :],
                 [0, 2, 4, 6])
    s1upool.release()
    s1pool.release()

    # ================= stage 2: block-diagonal dyn layer =================
    # contraction k order: deter block (4), xa (8), xs (8), xd (8) — matches
    # stage-1 completion order so stage 2 can start before W_d's LN drains.
    yssum = st_ps.tile([1, BC], F32, name="ssum")
    yssq = st_ps.tile([1, BC], F32, name="ssq")
    KT2 = ING // 128  # 28

    def rhs2(g, k):
        if k < 4:
            return deterB[:, g * 4 + k, :]
        if k < 12:
            return x_sb[:, 16 + (k - 4), :]
        if k < 20:
            return x_sb[:, 8 + (k - 12), :]
        return x_sb[:, k - 20, :]

    for g in range(G):
        psums = [mm_tile(j) for j in range(4)]
        for s in range(7):
            wp = wkpool.tile([128, 4, 512], BF16, name="wk")
            nc.sync.dma_start(
                out=wp.rearrange("p a b -> p (a b)"),
                in_=ins["dyn_P"][g, :, s * 2048:(s + 1) * 2048],
            )
            for k4 in range(4):
                k = s * 4 + k4
                for m in range(4):
                    nc.tensor.matmul(
                        psums[m],
                        wp[:, k4, m * 128:(m + 1) * 128],
                        rhs2(g, k),
                        start=(k == 0), stop=(k == KT2 - 1),
                    )
        for m in range(4):
            ft = g * 4 + m
            yt = y_u[:, ft, :]
            nc.vector.tensor_scalar_add(yt, psums[m],
                                        vecs_sb[:, C_DYNB + ft:C_DYNB + ft + 1])
            ysq = sqp.tile([128, BC], F32R, name="usq")
            nc.gpsimd.tensor_mul(ysq, yt, yt)
            nc.tensor.matmul(yssum, ones_col, yt,
                             start=(ft == 0), stop=(ft == NB_D - 1))
            nc.tensor.matmul(yssq, ones_col, ysq,
                             start=(ft == 0), stop=(ft == NB_D - 1))
    meanB2, rstdB2 = stats_finish(yssum, yssq, D)
    # bulk apply in stage-3 first-use order; Silu only -> one ACT table load
    _border = [0, 2, 5, 3, 6, 1, 4, 7]
    ln_apply(lambda i: y_u[:, i:i + 2, :].rearrange("p a b -> p (a b)"),
             meanB2, rstdB2, C_GDYN, C_BEDYN, lambda idx: y_bf[:, idx, :],
             [g * 4 + j for g in _border for j in (0, 2)])
    p3pool = tc.alloc_tile_pool(name="p3pool", bufs=2)
    outpool = tc.alloc_tile_pool(name="outpool", bufs=2)

    # ================= stage 3: GRU gates + output =================
    # zflat f-tiles: reset tj, cand tj+32, update tj+64; block gf = f*128//1536.
    # z chains rotate over 6 full PSUM banks (q0-3 + the dead bc0/bc1).
    zcnt = 0

    def z_slot():
        nonlocal zcnt
        i = zcnt % 6
        zcnt += 1
        if i < 4:
            return mm_ps.tile([128, 2 * BC], F32, name=f"q{i}")[:, :BC]
        return bc_ps.tile([128, 2 * BC], F32, name=f"bc{i - 4}")[:, :BC]

    def zmm(wp, gf, co):
        zp = z_slot()
        for k in range(4):
            nc.tensor.matmul(zp, wp[:, k, co:co + 128], y_bf[:, gf * 4 + k, :],
                             start=(k == 0), stop=(k == 3))
        return zp

    outst = None
    for seg in range(8):  # 4 tj per segment; each offset's 4 f-tiles in one block
        tj0 = seg * 4
        tjs = list(range(tj0, tj0 + 4))
        if seg % 2 == 0:
            outst = outpool.tile([128, 8, BC], F32, name="outst")
        panels = []
        for off in range(3):
            pidx = seg * 3 + off
            gf, _ = PANELS[pidx]
            wp = p3pool.tile([128, 4, 512], BF16, name=f"p{off}")
            nc.sync.dma_start(
                out=wp.rearrange("p a b -> p (a b)"),
                in_=ins["gru_P"][pidx],
            )
            panels.append((wp, gf))

        r, uu = {}, {}
        for tj in tjs:  # reset gate: sigmoid batch
            wp, gf = panels[0]
            zp = zmm(wp, gf, (tj - tj0) * 128)
            r[tj] = sqp.tile([128, BC], F32, name=f"r{tj % 2}")
            nc.scalar.activation(r[tj], zp, AF.Sigmoid,
                                 bias=vecs_sb[:, C_GRUB + tj:C_GRUB + tj + 1], scale=1.0)
        for tj in tjs:  # update gate: sigmoid batch
            wp, gf = panels[2]
            zp = zmm(wp, gf, (tj - tj0) * 128)
            uu[tj] = sqp.tile([128, BC], F32, name=f"uu{tj % 2}")
            nc.scalar.activation(uu[tj], zp, AF.Sigmoid,
                                 bias=vecs_sb[:, C_GRUB + 64 + tj:C_GRUB + 64 + tj + 1], scale=1.0)
        for tj in tjs:  # cand pre-act: rc = (z_c + b) * r in one DVE op
            wp, gf = panels[1]
            zp = zmm(wp, gf, (tj - tj0) * 128)
            rc = sqp.tile([128, BC], F32, name=f"rc{tj % 2}")
            nc.vector.scalar_tensor_tensor(
                rc, zp, vecs_sb[:, C_GRUB + 32 + tj:C_GRUB + 32 + tj + 1],
                r[tj], OP.add, OP.mult)
            ss = sqp.tile([128, BC], F32, name=f"ss{tj % 2}")
            nc.scalar.activation(ss, rc, AF.Sigmoid, bias=0.0, scale=2.0)
            dt_ = deterF[:, tj, :]
            cd = sqp.tile([128, BC], F32, name="cd")
            nc.vector.scalar_tensor_tensor(cd, ss, 2.0, dt_, OP.mult, OP.subtract)
            o = outst[:, tj % 8, :]
            nc.vector.scalar_tensor_tensor(o, cd, 1.0, uu[tj], OP.subtract, OP.mult)
            nc.gpsimd.tensor_add(o, o, dt_)
        if seg % 2 == 1:
            grp = seg // 2
            nc.sync.dma_start(
                out=outT[:, grp * 2048:(grp + 1) * 2048],
                in_=outst.rearrange("p a b -> p (a b)"))
    outpool.release()
    p3pool.release()


_CACHE = {}


def _build():
    if "nc" in _CACHE:
        return _CACHE["nc"]
    nc = bacc.Bacc("TRN2", target_bir_lowering=False, debug=False,
                   num_devices=NCORES)
    ins = {}
    for name, shape, dt in [
        ("deterT_r", [128, NB_D * BC], F32),
        ("stochT_b", [128, (S // 128) * BC], BF16),
        ("actionT_b", [128, 1 * BC], BF16),
        ("W_d_r", [128, NB_D * H], BF16),
        ("W_s_r", [128, (S // 128) * H], BF16),
        ("W_a_r", [128, 1 * H], BF16),
        ("dyn_P", [G, 128, 28 * 512], BF16),
        ("gru_P", [24, 128, 4 * 512], BF16),
        ("vecs", [128, NV], F32),
    ]:
        ins[name] = nc.dram_tensor(name, shape, dt, kind="ExternalInput").ap()
    outT = nc.dram_tensor("outT_r", [128, NB_D * BC], F32,
                          kind="ExternalOutput").ap()
    with tile.TileContext(nc) as tc:
        _emit(tc, ins, outT)
    nc.compile()
    _CACHE["nc"] = nc
    return nc


def _col_tile(v):
    """[L] -> [128, L//128] with col t holding v[t*128 + p]."""
    return np.ascontiguousarray(v.reshape(-1, 128).T.astype(np.float32))


def _make_vecs(b_d, g_d, be_d, b_s, g_s, be_s, b_a, g_a, be_a,
               dyn_b, g_dyn, be_dyn, gru_b):
    gru_adj = np.array(gru_b, dtype=np.float32).copy()
    gru_adj[2 * D:] -= 1.0
    cols = [b_d, g_d, be_d, b_s, g_s, be_s, b_a, g_a, be_a,
            dyn_b, g_dyn, be_dyn, gru_adj]
    return np.concatenate([_col_tile(np.asarray(c)) for c in cols], axis=1)


def _wtile(w, KT, N):
    """[KT*128, N] -> [128, KT*N] bf16: per partition, k-tiles contiguous."""
    w = np.asarray(w, np.float32).astype(NPBF)
    return np.ascontiguousarray(
        w.reshape(KT, 128, N).transpose(1, 0, 2).reshape(128, KT * N))


def _s1w(w, KT):
    """Stage-1 weight: [KT*128, 1024] -> [128, 2*KT*512] bf16, the two
    512-col output chunks laid out chunk-major then k-major."""
    w = np.asarray(w, np.float32)
    return np.concatenate([_wtile(w[:, c * 512:(c + 1) * 512], KT, 512)
                           for c in (0, 1)], axis=1)


def _atile(a, KT, dtype):
    """[B_slice, K] -> [128, KT*BC]: per partition, k-tiles contiguous."""
    t = np.ascontiguousarray(a.T).reshape(KT, 128, -1).transpose(1, 0, 2)
    return np.ascontiguousarray(t.reshape(128, -1).astype(dtype))


def kernel(deter, stoch, action,
           W_d, b_d, g_d, be_d,
           W_s, b_s, g_s, be_s,
           W_a, b_a, g_a, be_a,
           dyn_W, dyn_b, g_dyn, be_dyn,
           gru_W, gru_b):
    nc = _build()

    deter = np.asarray(deter, np.float32)
    stoch = np.asarray(stoch, np.float32)
    action = np.asarray(action, np.float32)
    vecs = _make_vecs(b_d, g_d, be_d, b_s, g_s, be_s, b_a, g_a, be_a,
                      dyn_b, g_dyn, be_dyn, gru_b)

    dW = np.asarray(dyn_W, np.float32)   # [G, ING, DG]
    # contraction row order: deter block, xa, xs, xd (see rhs2 in _emit)
    order = np.r_[0:512, 2560:3584, 1536:2560, 512:1536]
    dyn_P = np.stack([_wtile(dW[g][order], 28, 512) for g in range(G)])
    gW = np.asarray(gru_W, np.float32)   # [G, DG, 3*DG]
    gru_P = np.stack([_wtile(gW[gf][:, c0:c0 + 512], 4, 512)
                      for gf, c0 in PANELS])

    shared = {
        "W_d_r": _s1w(W_d, NB_D),
        "W_s_r": _s1w(W_s, S // 128),
        "W_a_r": _s1w(W_a, 1),
        "dyn_P": dyn_P,
        "gru_P": gru_P,
        "vecs": vecs,
    }
    in_maps = []
    for c in range(NCORES):
        sl = slice(c * BC, (c + 1) * BC)
        m = dict(shared)
        m["deterT_r"] = _atile(deter[sl], NB_D, np.float32)
        m["stochT_b"] = _atile(stoch[sl], S // 128, NPBF)
        m["actionT_b"] = _atile(action[sl], 1, NPBF)
        in_maps.append(m)

    import os
    kw = {}
    if os.environ.get("BASS_TMPDIR"):
        kw["tmpdir"] = os.environ["BASS_TMPDIR"]
    res = run_bass_kernel_spmd(nc, in_maps, list(range(NCORES)), **kw)
    global LAST_RES
    LAST_RES = res
    out = np.empty((B, D), np.float32)
    for c in range(NCORES):
        o = res.results[c]["outT_r"].reshape(128, NB_D, BC)
        out[c * BC:(c + 1) * BC] = o.transpose(2, 1, 0).reshape(BC, D)
    return out


LAST_RES = None


# revision 9
# speedup vs baseline: 1.3121x; 1.0584x over previous
"""BlockGRUCell fused Trainium2 kernel (bf16 matmul inputs).

Sharding: data-parallel over batch across 8 NeuronCores (weights replicated).
Dataflow is fully transposed ([features, batch] on-chip): every matmul uses
the weight matrix in natural [in, out] layout as the stationary lhsT and the
transposed activations [in, batch] as the moving rhs, producing the next
layer's transposed activations directly. All main matmuls run in bf16
(stationary weights bf16 -> Fast Weight Load hides LDWEIGHTS; halves HBM
traffic). LayerNorm stats and the GRU elementwise stay fp32. Weights are
pre-rearranged on the host into [partition, k, n] order so every DMA moves
multi-KB contiguous lines per partition.

PSUM discipline: at most ONE OPEN accumulation chain per bank (opening a
second chain in a bank wipes the first chain's partial sums). Closed results
survive a later chain opening in the same bank, so two sequential chains may
share a bank's halves — stage 3 exploits this to process tj pairs with
single [128, 512] ACT/DVE ops when the GRU biases are uniform (they are for
this model: 0 / 0 / -1), halving per-instruction overhead on the consumer
engines.
"""
import numpy as np
from contextlib import ExitStack

import ml_dtypes

import concourse.bass as bass
import concourse.tile as tile
from concourse import bacc, mybir
from concourse._compat import with_exitstack
from concourse.bass_utils import run_bass_kernel_spmd

B, D, S, H, G = 2048, 4096, 1024, 1024, 8
A = 128
DG = D // G            # 512
ING = DG + 3 * H       # 3584
NCORES = 8
BC = B // NCORES       # 256 batch rows per core
EPS = 1e-5
USE_SILU = True  # stage-1 only; CoreSim tests set False

F32 = mybir.dt.float32
F32R = mybir.dt.float32r
BF16 = mybir.dt.bfloat16
AF = mybir.ActivationFunctionType
OP = mybir.AluOpType
NPBF = ml_dtypes.bfloat16

# vecs column layout (all per-partition tiled: col j holds v[j*128 + p])
NB_H = H // 128        # 8
NB_D = D // 128        # 32
NB_Z = 3 * D // 128    # 96
C_BD, C_GD, C_BED = 0, NB_H, 2 * NB_H
C_BS, C_GS, C_BES = 3 * NB_H, 4 * NB_H, 5 * NB_H
C_BA, C_GA, C_BEA = 6 * NB_H, 7 * NB_H, 8 * NB_H
C_DYNB, C_GDYN, C_BEDYN = 9 * NB_H, 9 * NB_H + NB_D, 9 * NB_H + 2 * NB_D
C_GRUB = 9 * NB_H + 3 * NB_D
NV = C_GRUB + NB_Z     # 264

# stage-3 panel table: panel p = seg*3 + off covers z flat cols
# f0 = seg*512 + off*4096 .. +512 -> block gf = f0 // 1536, col0 = f0 % 1536
PANELS = []
for _seg in range(8):
    for _off in range(3):
        _f0 = _seg * 512 + _off * 4096
        PANELS.append((_f0 // 1536, _f0 % 1536))


@with_exitstack
def _emit(ctx: ExitStack, tc: tile.TileContext, ins: dict, outT: bass.AP,
          cfg: tuple):
    # cfg: (ln_uni, gru_uni) — ln_uni: per-LN (gamma, beta) floats or None,
    # keyed 'a','s','d','dyn'; gru_uni: (br, bc_, bu) floats or None.
    ln_uni, gru_uni = dict(cfg[0]), cfg[1]
    nc = tc.nc
    ctx.enter_context(nc.allow_low_precision(
        reason="bf16 matmul operands; fp32 accumulation and LN stats"))

    persist = ctx.enter_context(tc.tile_pool(name="persist", bufs=1))
    sqp = ctx.enter_context(tc.tile_pool(name="sqp", bufs=2))
    small = ctx.enter_context(tc.tile_pool(name="small", bufs=1))
    consts = ctx.enter_context(tc.tile_pool(name="consts", bufs=1))
    mm_ps = ctx.enter_context(tc.tile_pool(name="mm_ps", bufs=1, space="PSUM"))
    st_ps = ctx.enter_context(tc.tile_pool(name="st_ps", bufs=1, space="PSUM"))
    bc_ps = ctx.enter_context(tc.tile_pool(name="bc_ps", bufs=1, space="PSUM"))
    wkpool = ctx.enter_context(tc.tile_pool(name="wkpool", bufs=6))
    s1pool = tc.alloc_tile_pool(name="s1pool", bufs=1)
    s1upool = tc.alloc_tile_pool(name="s1upool", bufs=2)

    # ---- small constants ----
    vecs_sb = persist.tile([128, NV], F32, name="vecs")
    nc.sync.dma_start(out=vecs_sb, in_=ins["vecs"])
    ones_col_f = consts.tile([128, 1], F32)
    nc.vector.memset(ones_col_f, 1.0)
    ones_col = consts.tile([128, 1], F32R)
    nc.vector.tensor_copy(ones_col, ones_col_f)
    ones_row_f = consts.tile([1, 128], F32)
    nc.vector.memset(ones_row_f, 1.0)
    ones_row = consts.tile([1, 128], F32R)
    nc.vector.tensor_copy(ones_row, ones_row_f)
    eps_t = consts.tile([1, 1], F32)
    nc.vector.memset(eps_t, EPS)
    _ccols = {}

    def cbias(v):
        """ACT bias operand for a uniform constant: 0.0 passes through as
        an immediate; anything else becomes a memset [128, 1] column."""
        v = float(v)
        if v == 0.0:
            return 0.0
        if v not in _ccols:
            t = consts.tile([128, 1], F32, name=f"c{len(_ccols)}")
            nc.vector.memset(t, v)
            _ccols[v] = t
        return _ccols[v]

    # persistent activations
    x_sb = persist.tile([128, 24, BC], BF16, name="x")          # post-LN stage1
    deterF = persist.tile([128, NB_D, BC], F32, name="deterF")  # fp32 deter
    deterB = persist.tile([128, NB_D, BC], BF16, name="deterB")
    y_u = persist.tile([128, NB_D, BC], F32R, name="y_u")       # pre-LN stage2
    y_bf = persist.tile([128, NB_D, BC], BF16, name="y_bf")     # post-LN stage2

    def mm_tile(j):
        """One full PSUM bank per accumulation chain; left half carries data."""
        return mm_ps.tile([128, 2 * BC], F32, name=f"q{j}")[:, :BC]

    def stats_finish(ssum, ssq, nfeat):
        """Broadcast LN stats: returns (meanB2, rstdB2) [128, 2*BC] PSUM tiles
        with the per-batch stat duplicated in both halves (for paired apply)."""
        mr2 = small.tile([1, 4 * BC], F32R, name="mr2")
        mean, mean_b = mr2[:, :BC], mr2[:, BC:2 * BC]
        rstd, rstd_b = mr2[:, 2 * BC:3 * BC], mr2[:, 3 * BC:]
        nc.vector.tensor_scalar_mul(mean, ssum, 1.0 / nfeat)
        nc.vector.tensor_copy(mean_b, mean)
        m2 = small.tile([1, BC], F32, name="m2")
        nc.vector.tensor_mul(m2, mean, mean)
        var = small.tile([1, BC], F32, name="var")
        nc.vector.scalar_tensor_tensor(var, ssq, 1.0 / nfeat, m2,
                                       OP.mult, OP.subtract)
        std = small.tile([1, BC], F32, name="std")
        nc.scalar.activation(std, var, AF.Sqrt, bias=eps_t, scale=1.0)
        nc.vector.reciprocal(rstd, std)
        nc.vector.tensor_copy(rstd_b, rstd)
        bc0 = bc_ps.tile([128, 2 * BC], F32, name="bc0")
        nc.tensor.matmul(bc0, ones_row, mr2[:, :2 * BC], start=True, stop=True)
        bc1 = bc_ps.tile([128, 2 * BC], F32, name="bc1")
        nc.tensor.matmul(bc1, ones_row, mr2[:, 2 * BC:], start=True, stop=True)
        return bc0, bc1

    def ln_apply(u_sb_pair, meanB2, rstdB2, uni, g_cols, be_cols, dst_pair,
                 dst_half, pair_list):
        """Paired LN apply: dst = silu(((u - m) * r) * gamma + beta).

        u_sb_pair(i) -> [128, 2*BC] view of feature tiles (i, i+1); stats are
        read straight from the PSUM broadcast tiles. When gamma/beta are
        uniform scalars (uni != None) the silu runs on the whole pair."""
        for i in pair_list:
            tp = sqp.tile([128, 2 * BC], F32, name="tp")
            nc.vector.tensor_sub(tp, u_sb_pair(i), meanB2)
            nc.vector.tensor_mul(tp, tp, rstdB2)
            if uni is not None and USE_SILU:
                gv, bv = uni
                nc.scalar.activation(dst_pair(i), tp, AF.Silu,
                                     bias=cbias(bv), scale=float(gv))
                continue
            for h in (0, 1):
                idx = i + h
                gcol = vecs_sb[:, g_cols + idx:g_cols + idx + 1]
                becol = vecs_sb[:, be_cols + idx:be_cols + idx + 1]
                th = tp[:, h * BC:(h + 1) * BC]
                if USE_SILU:
                    nc.scalar.activation(dst_half(idx), th, AF.Silu,
                                         bias=becol, scale=gcol)
                else:
                    sg = sqp.tile([128, BC], F32, name="sg")
                    nc.scalar.activation(sg, th, AF.Sigmoid, bias=becol, scale=gcol)
                    nn = sqp.tile([128, BC], F32, name="nn")
                    nc.gpsimd.tensor_scalar(nn, th, gcol, becol, OP.mult, OP.add)
                    nc.gpsimd.tensor_mul(dst_half(idx), nn, sg)

    # ================= stage 1: three input projections =================
    def load_s1_rhs(wname):
        # emitted per-tensor so act loads sit just ahead of their weight
        # panels in the HWDGE FIFO (no head-of-line block of first matmuls)
        if wname == "W_a":
            t = s1pool.tile([128, 1, BC], BF16, name="actionT")
            nc.sync.dma_start(out=t.rearrange("p a b -> p (a b)"),
                              in_=ins["actionT_b"])
            # deter bf16 streams in behind the tiny action load, ahead of W_s
            for q in range(4):
                nc.sync.dma_start(
                    out=deterB[:, q * 8:(q + 1) * 8, :].rearrange("p a b -> p (a b)"),
                    in_=ins["deterT_bf"][:, q * 2048:(q + 1) * 2048])
        elif wname == "W_s":
            t = s1pool.tile([128, S // 128, BC], BF16, name="stochT")
            nc.sync.dma_start(out=t.rearrange("p a b -> p (a b)"),
                              in_=ins["stochT_b"])
        else:
            t = deterB
        return t

    stage1 = [
        ("W_a", 1, C_BA, C_GA, C_BEA, 16, 'a'),
        ("W_s", S // 128, C_BS, C_GS, C_BES, 8, 's'),
        ("W_d", NB_D, C_BD, C_GD, C_BED, 0, 'd'),
    ]
    for wname, KT, bcol, gcol, becol, xoff, lkey in stage1:
        Wsrc = ins[wname + "_r"]  # [128, 2*KT*512] bf16: chunk-major, k-major
        rhs_sb = load_s1_rhs(wname)
        u_sb = s1upool.tile([128, NB_H, BC], F32R, name="u_sb")
        ssum = st_ps.tile([1, BC], F32, name="ssum")
        ssq = st_ps.tile([1, BC], F32, name="ssq")
        for c in range(2):  # H in two chunks of 4 m-tiles
            psums = [mm_tile(j) for j in range(4)]
            KG = (KT + 3) // 4
            for kg in range(KG):
                nk = min(4, KT - kg * 4)
                wp = wkpool.tile([128, 4, 512], BF16, name="wk")
                base = c * KT * 512 + kg * 2048
                nc.sync.dma_start(
                    out=wp[:, :nk, :].rearrange("p a b -> p (a b)"),
                    in_=Wsrc[:, base:base + nk * 512],
                )
                for k4 in range(nk):
                    k = kg * 4 + k4
                    for m in range(4):
                        nc.tensor.matmul(
                            psums[m],
                            wp[:, k4, m * 128:(m + 1) * 128],
                            rhs_sb[:, k, :],
                            start=(k == 0), stop=(k == KT - 1),
                        )
            for m in range(4):
                mt = c * 4 + m
                ut = u_sb[:, mt, :]
                nc.vector.tensor_scalar_add(ut, psums[m],
                                            vecs_sb[:, bcol + mt:bcol + mt + 1])
                usq = sqp.tile([128, BC], F32R, name="usq")
                nc.gpsimd.tensor_mul(usq, ut, ut)
                nc.tensor.matmul(ssum, ones_col, ut,
                                 start=(mt == 0), stop=(mt == NB_H - 1))
                nc.tensor.matmul(ssq, ones_col, usq,
                                 start=(mt == 0), stop=(mt == NB_H - 1))
        meanB2, rstdB2 = stats_finish(ssum, ssq, H)
        ln_apply(lambda i, u=u_sb: u[:, i:i + 2, :].rearrange("p a b -> p (a b)"),
                 meanB2, rstdB2, ln_uni[lkey], gcol, becol,
                 lambda i, xo=xoff: x_sb[:, xo + i:xo + i + 2, :].rearrange(
                     "p a b -> p (a b)"),
                 lambda idx, xo=xoff: x_sb[:, xo + idx, :],
                 [0, 2, 4, 6])
        if wname == "W_s":
            # fp32 deter only feeds the stage-3 elementwise; stream it in the
            # quiet window behind W_d's weights
            for q in range(4):
                nc.sync.dma_start(
                    out=deterF[:, q * 8:(q + 1) * 8, :].rearrange("p a b -> p (a b)"),
                    in_=ins["deterT_r"][:, q * 2048:(q + 1) * 2048])
    s1upool.release()
    s1pool.release()

    # ================= stage 2: block-diagonal dyn layer =================
    # contraction k order: deter block (4), xa (8), xs (8), xd (8) — matches
    # stage-1 completion order so stage 2 can start before W_d's LN drains.
    yssum = st_ps.tile([1, BC], F32, name="ssum")
    yssq = st_ps.tile([1, BC], F32, name="ssq")
    KT2 = ING // 128  # 28

    def rhs2(g, k):
        if k < 4:
            return deterB[:, g * 4 + k, :]
        if k < 12:
            return x_sb[:, 16 + (k - 4), :]
        if k < 20:
            return x_sb[:, 8 + (k - 12), :]
        return x_sb[:, k - 20, :]

    for g in range(G):
        psums = [mm_tile(j) for j in range(4)]
        for s in range(7):
            wp = wkpool.tile([128, 4, 512], BF16, name="wk")
            nc.sync.dma_start(
                out=wp.rearrange("p a b -> p (a b)"),
                in_=ins["dyn_P"][g, :, s * 2048:(s + 1) * 2048],
            )
            for k4 in range(4):
                k = s * 4 + k4
                for m in range(4):
                    nc.tensor.matmul(
                        psums[m],
                        wp[:, k4, m * 128:(m + 1) * 128],
                        rhs2(g, k),
                        start=(k == 0), stop=(k == KT2 - 1),
                    )
        for m in range(4):
            ft = g * 4 + m
            yt = y_u[:, ft, :]
            nc.vector.tensor_scalar_add(yt, psums[m],
                                        vecs_sb[:, C_DYNB + ft:C_DYNB + ft + 1])
            ysq = sqp.tile([128, BC], F32R, name="usq")
            nc.gpsimd.tensor_mul(ysq, yt, yt)
            nc.tensor.matmul(yssum, ones_col, yt,
                             start=(ft == 0), stop=(ft == NB_D - 1))
            nc.tensor.matmul(yssq, ones_col, ysq,
                             start=(ft == 0), stop=(ft == NB_D - 1))
    meanB2, rstdB2 = stats_finish(yssum, yssq, D)
    # bulk apply in stage-3 first-use order; Silu only -> one ACT table load
    _border = [0, 2, 5, 3, 6, 1, 4, 7]
    ln_apply(lambda i: y_u[:, i:i + 2, :].rearrange("p a b -> p (a b)"),
             meanB2, rstdB2, ln_uni['dyn'], C_GDYN, C_BEDYN,
             lambda i: y_bf[:, i:i + 2, :].rearrange("p a b -> p (a b)"),
             lambda idx: y_bf[:, idx, :],
             [g * 4 + j for g in _border for j in (0, 2)])
    p3pool = tc.alloc_tile_pool(name="p3pool", bufs=2)
    outpool = tc.alloc_tile_pool(name="outpool", bufs=2)

    # ================= stage 3: GRU gates + output =================
    # zflat f-tiles: reset tj, cand tj+32, update tj+64; block gf = f*128//1536.
    # Paired path: each PSUM bank hosts the two SEQUENTIAL (closed) chains of
    # a tj pair; consumers run one [128, 512] op per pair. 6 banks rotate.
    zcnt = 0

    def z_bank():
        nonlocal zcnt
        i = zcnt % 6
        zcnt += 1
        if i < 4:
            return mm_ps.tile([128, 2 * BC], F32, name=f"q{i}")
        return bc_ps.tile([128, 2 * BC], F32, name=f"bc{i - 4}")

    def zmm_pair(wp, gf, co):
        zq = z_bank()
        for h in (0, 1):
            zp = zq[:, h * BC:(h + 1) * BC]
            for k in range(4):
                nc.tensor.matmul(zp, wp[:, k, co + h * 128:co + h * 128 + 128],
                                 y_bf[:, gf * 4 + k, :],
                                 start=(k == 0), stop=(k == 3))
        return zq

    def zmm_one(wp, gf, co):
        zp = z_bank()[:, :BC]
        for k in range(4):
            nc.tensor.matmul(zp, wp[:, k, co:co + 128], y_bf[:, gf * 4 + k, :],
                             start=(k == 0), stop=(k == 3))
        return zp

    outst = None
    for seg in range(8):  # 4 tj per segment; each offset's 4 f-tiles in one block
        tj0 = seg * 4
        if seg % 2 == 0:
            outst = outpool.tile([128, 8, BC], F32, name="outst")
        panels = []
        for off in range(3):
            pidx = seg * 3 + off
            gf, _ = PANELS[pidx]
            wp = p3pool.tile([128, 4, 512], BF16, name=f"p{off}")
            nc.sync.dma_start(
                out=wp.rearrange("p a b -> p (a b)"),
                in_=ins["gru_P"][pidx],
            )
            panels.append((wp, gf))

        if gru_uni is not None:
            br, bcv, bu = gru_uni
            r, uu = {}, {}
            for pr in (0, 1):  # reset gate pairs
                wp, gf = panels[0]
                zq = zmm_pair(wp, gf, pr * 256)
                r[pr] = sqp.tile([128, 2 * BC], F32, name=f"rp{pr}", bufs=1)
                nc.scalar.activation(r[pr], zq, AF.Sigmoid, bias=cbias(br),
                                     scale=1.0)
            for pr in (0, 1):  # update gate pairs
                wp, gf = panels[2]
                zq = zmm_pair(wp, gf, pr * 256)
                uu[pr] = sqp.tile([128, 2 * BC], F32, name=f"up{pr}", bufs=1)
                nc.scalar.activation(uu[pr], zq, AF.Sigmoid, bias=cbias(bu),
                                     scale=1.0)
            for pr in (0, 1):  # cand + output pairs
                wp, gf = panels[1]
                zq = zmm_pair(wp, gf, pr * 256)
                rc = sqp.tile([128, 2 * BC], F32, name=f"rcp{pr}", bufs=1)
                nc.vector.scalar_tensor_tensor(rc, zq, float(bcv), r[pr],
                                               OP.add, OP.mult)
                ss = sqp.tile([128, 2 * BC], F32, name=f"ssp{pr}", bufs=1)
                nc.scalar.activation(ss, rc, AF.Sigmoid, bias=0.0, scale=2.0)
                tjA = tj0 + 2 * pr
                dt_ = deterF[:, tjA:tjA + 2, :].rearrange("p a b -> p (a b)")
                cd = sqp.tile([128, 2 * BC], F32, name="cdp", bufs=1)
                nc.vector.scalar_tensor_tensor(cd, ss, 2.0, dt_, OP.mult,
                                               OP.subtract)
                o = outst[:, tjA % 8:tjA % 8 + 2, :].rearrange("p a b -> p (a b)")
                nc.vector.scalar_tensor_tensor(o, cd, 1.0, uu[pr],
                                               OP.subtract, OP.mult)
                nc.gpsimd.tensor_add(o, o, dt_)
        else:
            # general per-tj path (non-uniform GRU biases)
            tjs = list(range(tj0, tj0 + 4))
            r, uu = {}, {}
            for tj in tjs:
                wp, gf = panels[0]
                zp = zmm_one(wp, gf, (tj - tj0) * 128)
                r[tj] = sqp.tile([128, BC], F32, name=f"r{tj % 2}")
                nc.scalar.activation(r[tj], zp, AF.Sigmoid,
                                     bias=vecs_sb[:, C_GRUB + tj:C_GRUB + tj + 1],
                                     scale=1.0)
            for tj in tjs:
                wp, gf = panels[2]
                zp = zmm_one(wp, gf, (tj - tj0) * 128)
                uu[tj] = sqp.tile([128, BC], F32, name=f"uu{tj % 2}")
                nc.scalar.activation(uu[tj], zp, AF.Sigmoid,
                                     bias=vecs_sb[:, C_GRUB + 64 + tj:C_GRUB + 64 + tj + 1],
                                     scale=1.0)
            for tj in tjs:
                wp, gf = panels[1]
                zp = zmm_one(wp, gf, (tj - tj0) * 128)
                rc = sqp.tile([128, BC], F32, name=f"rc{tj % 2}")
                nc.vector.scalar_tensor_tensor(
                    rc, zp, vecs_sb[:, C_GRUB + 32 + tj:C_GRUB + 32 + tj + 1],
                    r[tj], OP.add, OP.mult)
                ss = sqp.tile([128, BC], F32, name=f"ss{tj % 2}")
                nc.scalar.activation(ss, rc, AF.Sigmoid, bias=0.0, scale=2.0)
                dt_ = deterF[:, tj, :]
                cd = sqp.tile([128, BC], F32, name="cd")
                nc.vector.scalar_tensor_tensor(cd, ss, 2.0, dt_, OP.mult,
                                               OP.subtract)
                o = outst[:, tj % 8, :]
                nc.vector.scalar_tensor_tensor(o, cd, 1.0, uu[tj],
                                               OP.subtract, OP.mult)
                nc.gpsimd.tensor_add(o, o, dt_)
        if seg % 2 == 1:
            grp = seg // 2
            nc.sync.dma_start(
                out=outT[:, grp * 2048:(grp + 1) * 2048],
                in_=outst.rearrange("p a b -> p (a b)"))
    outpool.release()
    p3pool.release()


_CACHE = {}


def _build(cfg):
    if cfg in _CACHE:
        return _CACHE[cfg]
    nc = bacc.Bacc("TRN2", target_bir_lowering=False, debug=False,
                   num_devices=NCORES)
    ins = {}
    for name, shape, dt in [
        ("deterT_r", [128, NB_D * BC], F32),
        ("deterT_bf", [128, NB_D * BC], BF16),
        ("stochT_b", [128, (S // 128) * BC], BF16),
        ("actionT_b", [128, 1 * BC], BF16),
        ("W_d_r", [128, NB_D * H], BF16),
        ("W_s_r", [128, (S // 128) * H], BF16),
        ("W_a_r", [128, 1 * H], BF16),
        ("dyn_P", [G, 128, 28 * 512], BF16),
        ("gru_P", [24, 128, 4 * 512], BF16),
        ("vecs", [128, NV], F32),
    ]:
        ins[name] = nc.dram_tensor(name, shape, dt, kind="ExternalInput").ap()
    outT = nc.dram_tensor("outT_r", [128, NB_D * BC], F32,
                          kind="ExternalOutput").ap()
    with tile.TileContext(nc) as tc:
        _emit(tc, ins, outT, cfg)
    nc.compile()
    _CACHE[cfg] = nc
    return nc


def _col_tile(v):
    """[L] -> [128, L//128] with col t holding v[t*128 + p]."""
    return np.ascontiguousarray(v.reshape(-1, 128).T.astype(np.float32))


def _make_vecs(b_d, g_d, be_d, b_s, g_s, be_s, b_a, g_a, be_a,
               dyn_b, g_dyn, be_dyn, gru_adj):
    cols = [b_d, g_d, be_d, b_s, g_s, be_s, b_a, g_a, be_a,
            dyn_b, g_dyn, be_dyn, gru_adj]
    return np.concatenate([_col_tile(np.asarray(c)) for c in cols], axis=1)


def _uni(v):
    v = np.asarray(v, np.float32)
    return float(v.flat[0]) if np.all(v == v.flat[0]) else None


def _wtile(w, KT, N):
    """[KT*128, N] -> [128, KT*N] bf16: per partition, k-tiles contiguous."""
    w = np.asarray(w, np.float32).astype(NPBF)
    return np.ascontiguousarray(
        w.reshape(KT, 128, N).transpose(1, 0, 2).reshape(128, KT * N))


def _s1w(w, KT):
    """Stage-1 weight: [KT*128, 1024] -> [128, 2*KT*512] bf16, the two
    512-col output chunks laid out chunk-major then k-major."""
    w = np.asarray(w, np.float32)
    return np.concatenate([_wtile(w[:, c * 512:(c + 1) * 512], KT, 512)
                           for c in (0, 1)], axis=1)


def _atile(a, KT, dtype):
    """[B_slice, K] -> [128, KT*BC]: per partition, k-tiles contiguous."""
    t = np.ascontiguousarray(a.T).reshape(KT, 128, -1).transpose(1, 0, 2)
    return np.ascontiguousarray(t.reshape(128, -1).astype(dtype))


def kernel(deter, stoch, action,
           W_d, b_d, g_d, be_d,
           W_s, b_s, g_s, be_s,
           W_a, b_a, g_a, be_a,
           dyn_W, dyn_b, g_dyn, be_dyn,
           gru_W, gru_b):
    gru_adj = np.array(gru_b, dtype=np.float32).copy()
    gru_adj[2 * D:] -= 1.0
    ln_uni = tuple(sorted({
        'a': None if (u := _uni(g_a)) is None or (v := _uni(be_a)) is None else (u, v),
        's': None if (u := _uni(g_s)) is None or (v := _uni(be_s)) is None else (u, v),
        'd': None if (u := _uni(g_d)) is None or (v := _uni(be_d)) is None else (u, v),
        'dyn': None if (u := _uni(g_dyn)) is None or (v := _uni(be_dyn)) is None else (u, v),
    }.items()))
    br, bcv, bu = (_uni(gru_adj[:D]), _uni(gru_adj[D:2 * D]),
                   _uni(gru_adj[2 * D:]))
    gru_uni = None if br is None or bcv is None or bu is None else (br, bcv, bu)
    cfg = (ln_uni, gru_uni)
    nc = _build(cfg)

    deter = np.asarray(deter, np.float32)
    stoch = np.asarray(stoch, np.float32)
    action = np.asarray(action, np.float32)
    vecs = _make_vecs(b_d, g_d, be_d, b_s, g_s, be_s, b_a, g_a, be_a,
                      dyn_b, g_dyn, be_dyn, gru_adj)

    dW = np.asarray(dyn_W, np.float32)   # [G, ING, DG]
    # contraction row order: deter block, xa, xs, xd (see rhs2 in _emit)
    order = np.r_[0:512, 2560:3584, 1536:2560, 512:1536]
    dyn_P = np.stack([_wtile(dW[g][order], 28, 512) for g in range(G)])
    gW = np.asarray(gru_W, np.float32)   # [G, DG, 3*DG]
    gru_P = np.stack([_wtile(gW[gf][:, c0:c0 + 512], 4, 512)
                      for gf, c0 in PANELS])

    shared = {
        "W_d_r": _s1w(W_d, NB_D),
        "W_s_r": _s1w(W_s, S // 128),
        "W_a_r": _s1w(W_a, 1),
        "dyn_P": dyn_P,
        "gru_P": gru_P,
        "vecs": vecs,
    }
    in_maps = []
    for c in range(NCORES):
        sl = slice(c * BC, (c + 1) * BC)
        m = dict(shared)
        m["deterT_r"] = _atile(deter[sl], NB_D, np.float32)
        m["deterT_bf"] = _atile(deter[sl], NB_D, NPBF)
        m["stochT_b"] = _atile(stoch[sl], S // 128, NPBF)
        m["actionT_b"] = _atile(action[sl], 1, NPBF)
        in_maps.append(m)

    import os
    kw = {}
    if os.environ.get("BASS_TMPDIR"):
        kw["tmpdir"] = os.environ["BASS_TMPDIR"]
    res = run_bass_kernel_spmd(nc, in_maps, list(range(NCORES)), **kw)
    global LAST_RES
    LAST_RES = res
    out = np.empty((B, D), np.float32)
    for c in range(NCORES):
        o = res.results[c]["outT_r"].reshape(128, NB_D, BC)
        out[c * BC:(c + 1) * BC] = o.transpose(2, 1, 0).reshape(BC, D)
    return out


LAST_RES = None
